# revision 12
# baseline (speedup 1.0000x reference)
"""Mamba block kernel for Trainium2, 8 NeuronCores.

Sharding: DP-2 over batch x TP-4 over d_inner (512 channels/core).
Core c = b*4 + g handles batch b, channels [g*512, (g+1)*512).

Per-core pipeline (everything in transposed [feature, time] layout):
  A) stats (mean/var of x over d_model via PE ones-reduce) + in_proj matmul
     with LN folded in (rank-1 mu correction in PSUM, rstd scaling fused
     into the PSUM->SBUF evacuation).
  B) depthwise causal conv (ACT per-tap scale + PE identity-accumulate),
     SiLU, x_proj partial + AllReduce(group of 4), dt_proj + softplus, dt*u.
  C) selective scan: per (n, d-tile): dA = exp(A_n * dt) on ACT,
     dBu = dtu * B_bcast on DVE, h = tensor_tensor_scan on DVE (full L),
     hC = h * C_bcast on GPSIMD, n-accumulate + Dskip*u_c via PE
     identity/diag matmuls into PSUM, gate with silu(z) fused into evac.
  D) out_proj partial matmul -> transposed partial output to HBM.

Host: preps transposed/bf16 weights, sums the 4 TP partials per batch,
adds the residual.
"""

import numpy as np
import ml_dtypes

D_MODEL, D_STATE, D_CONV, EXPAND = 1024, 16, 4, 2
D_INNER = EXPAND * D_MODEL            # 2048
DT_RANK = 64
B, L = 2, 2048
EPS = 1e-5
N_CORES = 8
TP = 4                                # TP group size
DP = D_INNER // TP                    # 512 channels per core
NDT = DP // 128                       # 4 d-tiles per core
BF16 = ml_dtypes.bfloat16

_CACHE = {}


def _build_program():
    import concourse.bass as bass
    import concourse.tile as tile
    from concourse import bacc, mybir

    F32, BF = mybir.dt.float32, mybir.dt.bfloat16
    ALU = mybir.AluOpType
    ACT = mybir.ActivationFunctionType

    nc = bacc.Bacc("TRN2", target_bir_lowering=False, debug=False,
                   num_devices=N_CORES)

    # ---- per-core external tensors ----
    xT = nc.dram_tensor("xT", [D_MODEL, L], BF, kind="ExternalInput")
    winT = nc.dram_tensor("winT", [D_MODEL, 2 * DP], BF, kind="ExternalInput")
    negrs = nc.dram_tensor("negrs", [1, 2 * DP], BF, kind="ExternalInput")
    c0rs = nc.dram_tensor("c0rs", [1, 2 * DP], BF, kind="ExternalInput")
    convw = nc.dram_tensor("convw", [DP, D_CONV], F32, kind="ExternalInput")
    convb = nc.dram_tensor("convb", [DP, 1], F32, kind="ExternalInput")
    xpwT = nc.dram_tensor("xpwT", [DP, DT_RANK + 2 * D_STATE], BF, kind="ExternalInput")
    dtwT = nc.dram_tensor("dtwT", [DT_RANK, DP], BF, kind="ExternalInput")
    dtb = nc.dram_tensor("dtb", [DP, 1], F32, kind="ExternalInput")
    Aneg = nc.dram_tensor("Aneg", [DP, D_STATE], F32, kind="ExternalInput")
    dskd = nc.dram_tensor("dskd", [DP, 128], BF, kind="ExternalInput")
    ident_in = nc.dram_tensor("ident", [128, 128], BF, kind="ExternalInput")
    owT = nc.dram_tensor("owT", [DP, D_MODEL], BF, kind="ExternalInput")
    out = nc.dram_tensor("out", [D_MODEL, L], F32, kind="ExternalOutput")

    NK = D_MODEL // 128               # 8 k-chunks
    NM = (2 * DP) // 128              # 8 m-chunks of in_proj output
    NTC = L // 512                    # 4 t-chunks of 512

    with tile.TileContext(nc) as tc:
        with tc.tile_pool(name="persist", bufs=1) as pp, \
             tc.tile_pool(name="dram", bufs=1, space="DRAM") as dram:

            # persistent SBUF tiles
            uT = [pp.tile([128, L + 4], BF, tag=f"uT{i}", name=f"uT{i}") for i in range(NDT)]
            zT = [pp.tile([128, L], BF, tag=f"zT{i}", name=f"zT{i}") for i in range(NDT)]
            ucT = [pp.tile([128, L], BF, tag=f"ucT{i}", name=f"ucT{i}") for i in range(NDT)]
            szT = [pp.tile([128, L], BF, tag=f"szT{i}", name=f"szT{i}") for i in range(NDT)]
            dtT = [pp.tile([128, L], BF, tag=f"dtT{i}", name=f"dtT{i}") for i in range(NDT)]
            dtuT = [pp.tile([128, L], BF, tag=f"dtuT{i}", name=f"dtuT{i}") for i in range(NDT)]
            ysg = [pp.tile([128, L], BF, tag=f"ysg{i}", name=f"ysg{i}") for i in range(NDT)]
            ident = pp.tile([128, 128], BF, tag="ident")
            nc.sync.dma_start(ident[:], ident_in.ap())
            dskd_sb, Aneg_sb, convw_sb, convb_sb, dtb_sb = [], [], [], [], []
            for i in range(NDT):
                rsl = slice(i * 128, (i + 1) * 128)
                t = pp.tile([128, 128], BF, name=f"dskd{i}")
                nc.sync.dma_start(t[:], dskd.ap()[rsl, :]); dskd_sb.append(t)
                t = pp.tile([128, D_STATE], F32, name=f"Aneg{i}")
                nc.sync.dma_start(t[:], Aneg.ap()[rsl, :]); Aneg_sb.append(t)
                t = pp.tile([128, D_CONV], F32, name=f"convw{i}")
                nc.sync.dma_start(t[:], convw.ap()[rsl, :]); convw_sb.append(t)
                t = pp.tile([128, 1], F32, name=f"convb{i}")
                nc.sync.dma_start(t[:], convb.ap()[rsl, :]); convb_sb.append(t)
                t = pp.tile([128, 1], F32, name=f"dtb{i}")
                nc.sync.dma_start(t[:], dtb.ap()[rsl, :]); dtb_sb.append(t)
            negrs_sb = pp.tile([1, 2 * DP], BF, tag="negrs")
            nc.sync.dma_start(negrs_sb[:], negrs.ap())
            c0rs_sb = pp.tile([1, 2 * DP], BF, tag="c0rs")
            nc.sync.dma_start(c0rs_sb[:], c0rs.ap())

            # DRAM scratch
            xdbl_part = dram.tile([DT_RANK + 2 * D_STATE, L], F32, tag="xdp")
            xdbl_red = dram.tile([DT_RANK + 2 * D_STATE, L], F32, tag="xdr")
            bc16 = dram.tile([2 * D_STATE, L], BF, tag="bc16")
            rstd_dram = dram.tile([1, L], F32, tag="rstdd")

            # zero the conv pad columns
            for i in range(NDT):
                nc.vector.memset(uT[i][:, 0:4], 0.0)

            # ---------- Phase A: load xT, stats, in_proj ----------
            with tc.tile_pool(name="xk", bufs=1) as xkp, \
                 tc.tile_pool(name="stat", bufs=1) as statp:
                xk = []
                for kc in range(NK):
                    t = xkp.tile([128, L], BF, tag=f"xk{kc}")
                    nc.sync.dma_start(t[:], xT.ap()[kc * 128:(kc + 1) * 128, :])
                    xk.append(t)

                ones = pp.tile([128, 1], BF, tag="ones")
                nc.vector.memset(ones[:], 1.0)

                # stats: S1 = colsum(x), S2 = colsum(x^2) via PE ones-reduce
                with tc.tile_pool(name="stps", bufs=1, space="PSUM") as stps, \
                     tc.tile_pool(name="sq", bufs=2) as sqp:
                    S1 = stps.tile([1, L], F32, tag="S1")
                    S2 = stps.tile([1, L], F32, tag="S2")
                    for kc in range(NK):
                        x2 = sqp.tile([128, L], BF, tag="x2")
                        nc.scalar.activation(x2[:], xk[kc][:], ACT.Square)
                        for t4 in range(NTC):
                            sl = slice(t4 * 512, (t4 + 1) * 512)
                            nc.tensor.matmul(S1[:, sl], ones[:], xk[kc][:, sl],
                                             start=(kc == 0), stop=(kc == NK - 1))
                            nc.tensor.matmul(S2[:, sl], ones[:], x2[:, sl],
                                             start=(kc == 0), stop=(kc == NK - 1))
                    # -> SBUF
                    s1f = statp.tile([1, L], F32, tag="s1f")
                    s2f = statp.tile([1, L], F32, tag="st", bufs=3)
                    nc.scalar.activation(s1f[:], S1[:], ACT.Copy)
                    nc.scalar.activation(s2f[:], S2[:], ACT.Copy)

                # rstd = exp(-0.5*log(var+eps)); sd = exp(+0.5*log(var+eps))
                mu2 = statp.tile([1, L], F32, tag="st", bufs=3)
                nc.scalar.activation(mu2[:], s1f[:], ACT.Square, scale=1.0 / D_MODEL)
                var = statp.tile([1, L], F32, tag="st", bufs=3)
                nc.vector.scalar_tensor_tensor(var[:], s2f[:], 1.0 / D_MODEL, mu2[:],
                                               ALU.mult, ALU.subtract)
                epsb = statp.tile([1, 1], F32, tag="epsb")
                nc.vector.memset(epsb[:], EPS)
                lv = statp.tile([1, L], F32, tag="st", bufs=3)
                nc.scalar.activation(lv[:], var[:], ACT.Ln, bias=epsb[:])
                rstd_row = statp.tile([1, L], F32, tag="st", bufs=3)
                nc.scalar.activation(rstd_row[:], lv[:], ACT.Exp, scale=-0.5)
                sd16 = pp.tile([1, L], BF, tag="sd16")
                nc.scalar.activation(sd16[:], lv[:], ACT.Exp, scale=0.5)
                s1_16 = pp.tile([1, L], BF, tag="s1_16")
                nc.vector.tensor_copy(s1_16[:], s1f[:])
                # broadcast rstd to all partitions via DRAM bounce
                nc.sync.dma_start(rstd_dram[:], rstd_row[:])
                rstd_bc = pp.tile([128, L], F32, tag="rstbc")
                nc.sync.dma_start(rstd_bc[:], rstd_dram[0, :].partition_broadcast(128))

                # in_proj
                with tc.tile_pool(name="wln", bufs=4) as wp, \
                     tc.tile_pool(name="xzps", bufs=2, space="PSUM") as xzps:
                    for mc in range(NM):
                        ps = xzps.tile([128, L], F32, tag="xz")
                        for kc in range(NK):
                            w = wp.tile([128, 128], BF, tag="w")
                            nc.sync.dma_start(
                                w[:], winT.ap()[kc * 128:(kc + 1) * 128,
                                                mc * 128:(mc + 1) * 128])
                            for t4 in range(NTC):
                                sl = slice(t4 * 512, (t4 + 1) * 512)
                                nc.tensor.matmul(ps[:, sl], w[:], xk[kc][:, sl],
                                                 start=(kc == 0), stop=False)
                        for t4 in range(NTC):
                            sl = slice(t4 * 512, (t4 + 1) * 512)
                            nc.tensor.matmul(ps[:, sl],
                                             negrs_sb[:, mc * 128:(mc + 1) * 128],
                                             s1_16[:, sl], start=False, stop=False)
                            nc.tensor.matmul(ps[:, sl],
                                             c0rs_sb[:, mc * 128:(mc + 1) * 128],
                                             sd16[:, sl], start=False, stop=True)
                        if mc < NDT:
                            nc.vector.tensor_tensor(uT[mc][:, 4:4 + L], ps[:],
                                                    rstd_bc[:], ALU.mult)
                        else:
                            nc.vector.tensor_tensor(zT[mc - NDT][:], ps[:],
                                                    rstd_bc[:], ALU.mult)

            # ---------- Phase B ----------
            # conv: 4 shifted taps scaled on ACT, summed on PE, SiLU evac
            with tc.tile_pool(name="cvps", bufs=2, space="PSUM") as cvps, \
                 tc.tile_pool(name="taps", bufs=2) as tapp:
                for i in range(NDT):
                    psc = cvps.tile([128, L], F32, tag="cv")
                    cks = []
                    for k in range(D_CONV):
                        ck = tapp.tile([128, L], BF, tag=f"ck{k}")
                        nc.scalar.activation(ck[:], uT[i][:, 1 + k:1 + k + L],
                                             ACT.Copy,
                                             scale=convw_sb[i][:, k:k + 1])
                        cks.append(ck)
                    for t4 in range(NTC):
                        sl = slice(t4 * 512, (t4 + 1) * 512)
                        for k in range(D_CONV):
                            nc.tensor.matmul(psc[:, sl], ident[:], cks[k][:, sl],
                                             start=(k == 0), stop=(k == D_CONV - 1))
                    nc.scalar.activation(ucT[i][:], psc[:], ACT.Silu,
                                         bias=convb_sb[i][:])
                    # silu(z) too
                    nc.scalar.activation(szT[i][:], zT[i][:], ACT.Silu)
            # x_proj partial + AllReduce
            NXP = DT_RANK + 2 * D_STATE
            with tc.tile_pool(name="xpps", bufs=1, space="PSUM") as xpps, \
                 tc.tile_pool(name="xpw", bufs=1) as xpwp:
                psx = xpps.tile([NXP, L], F32, tag="xp")
                xpw_sb = []
                for i in range(NDT):
                    t = xpwp.tile([128, NXP], BF, name=f"xpw{i}")
                    nc.sync.dma_start(t[:], xpwT.ap()[i * 128:(i + 1) * 128, :])
                    xpw_sb.append(t)
                for i in range(NDT):
                    for t4 in range(NTC):
                        sl = slice(t4 * 512, (t4 + 1) * 512)
                        nc.tensor.matmul(psx[:, sl], xpw_sb[i][:],
                                         ucT[i][:, sl],
                                         start=(i == 0), stop=(i == NDT - 1))
                xdbl_sb = xpwp.tile([NXP, L], F32, tag="xdbl")
                nc.scalar.activation(xdbl_sb[:], psx[:], ACT.Copy)
                nc.sync.dma_start(xdbl_part[:], xdbl_sb[:])

            nc.gpsimd.collective_compute(
                "AllReduce", ALU.add,
                replica_groups=[[0, 1, 2, 3], [4, 5, 6, 7]],
                ins=[xdbl_part[:].opt()],
                outs=[xdbl_red[:].opt()],
            )

            # dtr (bf16) and B/C rows (bf16, bounced to DRAM for broadcasts)
            with tc.tile_pool(name="dtr", bufs=1) as dtrp:
                dtr_f = dtrp.tile([DT_RANK, L], F32, tag="dtrf")
                nc.sync.dma_start(dtr_f[:], xdbl_red[0:DT_RANK, :])
                dtr16 = dtrp.tile([DT_RANK, L], BF, tag="dtr16")
                nc.vector.tensor_copy(dtr16[:], dtr_f[:])
                bc_f = dtrp.tile([2 * D_STATE, L], F32, tag="bcf")
                nc.sync.dma_start(bc_f[:], xdbl_red[DT_RANK:NXP, :])
                bc_sb = dtrp.tile([2 * D_STATE, L], BF, tag="bcs")
                nc.vector.tensor_copy(bc_sb[:], bc_f[:])
                nc.sync.dma_start(bc16[:], bc_sb[:])

                # dt_proj + softplus + dtu
                with tc.tile_pool(name="dtps", bufs=2, space="PSUM") as dtps, \
                     tc.tile_pool(name="dtw", bufs=1) as dtwp:
                    dtw_sb = dtwp.tile([DT_RANK, DP], BF, tag="dtw")
                    nc.sync.dma_start(dtw_sb[:], dtwT.ap())
                    etile = dtwp.tile([128, L], F32, tag="et")
                    for i in range(NDT):
                        psd = dtps.tile([128, L], F32, tag="dt")
                        for t4 in range(NTC):
                            sl = slice(t4 * 512, (t4 + 1) * 512)
                            nc.tensor.matmul(psd[:, sl],
                                             dtw_sb[:, i * 128:(i + 1) * 128],
                                             dtr16[:, sl], start=True, stop=True)
                        dsl = slice(i * 128, (i + 1) * 128)
                        nc.scalar.activation(etile[:], psd[:], ACT.Exp,
                                             bias=dtb_sb[i][:])
                        nc.scalar.activation(dtT[i][:], etile[:], ACT.Ln, bias=1.0)
                        nc.vector.tensor_tensor(dtuT[i][:], dtT[i][:],
                                                ucT[i][:], ALU.mult)

            # ---------- Phase C: selective scan ----------
            with tc.tile_pool(name="ysps", bufs=1, space="PSUM") as ysps, \
                 tc.tile_pool(name="scw", bufs=2) as scw, \
                 tc.tile_pool(name="bcw", bufs=2) as bcw:
                for pair in range(NDT // 2):
                    ys = []
                    for j in range(2):
                        i = pair * 2 + j
                        ps = ysps.tile([128, L], F32, tag=f"ys{j}")
                        for t4 in range(NTC):
                            sl = slice(t4 * 512, (t4 + 1) * 512)
                            nc.tensor.matmul(ps[:, sl], dskd_sb[i][:],
                                             ucT[i][:, sl], start=True, stop=False)
                        ys.append(ps)
                    for n in range(D_STATE):
                        bbc = bcw.tile([128, L], BF, tag="bbc")
                        nc.sync.dma_start(bbc[:], bc16[n, :].partition_broadcast(128))
                        cbc = bcw.tile([128, L], BF, tag="cbc")
                        nc.sync.dma_start(
                            cbc[:], bc16[D_STATE + n, :].partition_broadcast(128))
                        for j in range(2):
                            i = pair * 2 + j
                            dsl = slice(i * 128, (i + 1) * 128)
                            dA = scw.tile([128, L], F32, tag="dA")
                            nc.scalar.activation(dA[:], dtT[i][:], ACT.Exp,
                                                 scale=Aneg_sb[i][:, n:n + 1])
                            dBu = scw.tile([128, L], BF, tag="dBu", bufs=3)
                            nc.gpsimd.tensor_tensor(dBu[:], dtuT[i][:], bbc[:],
                                                    ALU.mult)
                            h = scw.tile([128, L], BF, tag="h", bufs=3)
                            nc.vector.tensor_tensor_scan(h[:], dA[:], dBu[:], 0.0,
                                                         ALU.mult, ALU.add)
                            hC = scw.tile([128, L], BF, tag="hC", bufs=3)
                            nc.vector.tensor_tensor(hC[:], h[:], cbc[:], ALU.mult)
                            last = (n == D_STATE - 1)
                            for t4 in range(NTC):
                                sl = slice(t4 * 512, (t4 + 1) * 512)
                                nc.tensor.matmul(ys[j][:, sl], ident[:],
                                                 hC[:, sl], start=False, stop=last)
                    for j in range(2):
                        i = pair * 2 + j
                        nc.vector.tensor_tensor(ysg[i][:], ys[j][:], szT[i][:],
                                                ALU.mult)

            # ---------- Phase D: out_proj ----------
            with tc.tile_pool(name="ops", bufs=4, space="PSUM") as ops, \
                 tc.tile_pool(name="ow", bufs=1) as owp, \
                 tc.tile_pool(name="oev", bufs=4) as oevp:
                ow_sb = []
                for i in range(NDT):
                    t = owp.tile([128, D_MODEL], BF, name=f"ow{i}")
                    nc.sync.dma_start(t[:], owT.ap()[i * 128:(i + 1) * 128, :])
                    ow_sb.append(t)
                for mc in range(D_MODEL // 128):
                    for t4 in range(NTC):
                        sl = slice(t4 * 512, (t4 + 1) * 512)
                        po = ops.tile([128, 512], F32, tag="po")
                        for i in range(NDT):
                            nc.tensor.matmul(po[:],
                                             ow_sb[i][:, mc * 128:(mc + 1) * 128],
                                             ysg[i][:, sl],
                                             start=(i == 0), stop=(i == NDT - 1))
                        oe = oevp.tile([128, 512], F32, tag="oe")
                        nc.scalar.activation(oe[:], po[:], ACT.Copy)
                        nc.sync.dma_start(
                            out.ap()[mc * 128:(mc + 1) * 128, sl], oe[:])

    nc.compile()
    return nc


def _prep_inputs(x, ln_w, ln_b, in_proj_w, conv_w, conv_b, x_proj_w,
                 dt_proj_w, dt_proj_b, A_log, Dskip, out_proj_w):
    """Host-side shard + transpose + dtype prep. Returns list of 8 in_maps."""
    f32 = np.float32
    x = np.asarray(x, f32)
    ln_w = np.asarray(ln_w, f32); ln_b = np.asarray(ln_b, f32)
    W = np.asarray(in_proj_w, f32)
    W_eff = W * ln_w[None, :]
    c0 = W @ ln_b                                  # [2*D_INNER]
    rs = W_eff.sum(axis=1)                         # [2*D_INNER]
    A = -np.exp(np.asarray(A_log, f32))            # [D_INNER, 16]
    conv_w = np.asarray(conv_w, f32).reshape(D_INNER, D_CONV)
    conv_b = np.asarray(conv_b, f32)
    xpw = np.asarray(x_proj_w, f32)                # [96, D_INNER]
    dtw = np.asarray(dt_proj_w, f32)               # [D_INNER, 64]
    dtb = np.asarray(dt_proj_b, f32)
    Dsk = np.asarray(Dskip, f32)
    Ow = np.asarray(out_proj_w, f32)               # [D_MODEL, D_INNER]
    ident = np.eye(128, dtype=BF16)

    in_maps = []
    for c in range(N_CORES):
        b, g = divmod(c, TP)
        dsl = slice(g * DP, (g + 1) * DP)
        u_rows = slice(g * DP, (g + 1) * DP)
        z_rows = slice(D_INNER + g * DP, D_INNER + (g + 1) * DP)
        winT = np.concatenate([W_eff[u_rows].T, W_eff[z_rows].T], axis=1)  # [1024, 1024]
        negrs_c = -np.concatenate([rs[u_rows], rs[z_rows]]) / D_MODEL
        c0_c = np.concatenate([c0[u_rows], c0[z_rows]])
        dskd = np.zeros((DP, 128), BF16)
        for i in range(NDT):
            blk = np.diag(Dsk[g * DP + i * 128: g * DP + (i + 1) * 128])
            dskd[i * 128:(i + 1) * 128, :] = blk.astype(BF16)
        in_maps.append({
            "xT": np.ascontiguousarray(x[b].T).astype(BF16),
            "winT": winT.astype(BF16),
            "negrs": negrs_c[None, :].astype(BF16),
            "c0rs": c0_c[None, :].astype(BF16),
            "convw": np.ascontiguousarray(conv_w[dsl]),
            "convb": conv_b[dsl][:, None].copy(),
            "xpwT": np.ascontiguousarray(xpw[:, dsl].T).astype(BF16),
            "dtwT": np.ascontiguousarray(dtw[dsl].T).astype(BF16),
            "dtb": dtb[dsl][:, None].copy(),
            "Aneg": np.ascontiguousarray(A[dsl]),
            "dskd": dskd,
            "ident": ident,
            "owT": np.ascontiguousarray(Ow[:, dsl].T).astype(BF16),
        })
    return in_maps


def kernel(**inputs):
    from concourse.bass_utils import run_bass_kernel_spmd

    if "nc" not in _CACHE:
        _CACHE["nc"] = _build_program()
    nc = _CACHE["nc"]

    in_maps = _prep_inputs(**inputs)
    res = run_bass_kernel_spmd(nc, in_maps, core_ids=list(range(N_CORES)))

    x = np.asarray(inputs["x"], np.float32)
    out = np.empty((B, L, D_MODEL), np.float32)
    for b in range(B):
        acc = res.results[4 * b]["out"].copy()
        for g in range(1, TP):
            acc += res.results[4 * b + g]["out"]
        out[b] = acc.T + x[b]
    return out


# revision 13
# speedup vs baseline: 1.2115x; 1.2115x over previous
"""Mamba block kernel for Trainium2, 8 NeuronCores.

Sharding: DP-2 over batch x TP-4 over d_inner (512 channels/core).
Core c = b*4 + g handles batch b, channels [g*512, (g+1)*512).

Per-core pipeline (everything in transposed [feature, time] layout):
  A) stats (mean/var of x over d_model via PE ones-reduce) + in_proj matmul
     with LN folded in (rank-1 mu correction in PSUM, rstd scaling fused
     into the PSUM->SBUF evacuation).
  B) depthwise causal conv (ACT per-tap scale + PE identity-accumulate),
     SiLU, x_proj partial + AllReduce(group of 4), dt_proj + softplus, dt*u.
  C) selective scan: per (n, d-tile): dA = exp(A_n * dt) on ACT,
     dBu = dtu * B_bcast on DVE, h = tensor_tensor_scan on DVE (full L),
     hC = h * C_bcast on GPSIMD, n-accumulate + Dskip*u_c via PE
     identity/diag matmuls into PSUM, gate with silu(z) fused into evac.
  D) out_proj partial matmul -> transposed partial output to HBM.

Host: preps transposed/bf16 weights, sums the 4 TP partials per batch,
adds the residual.
"""

import numpy as np
import ml_dtypes

D_MODEL, D_STATE, D_CONV, EXPAND = 1024, 16, 4, 2
D_INNER = EXPAND * D_MODEL            # 2048
DT_RANK = 64
B, L = 2, 2048
EPS = 1e-5
N_CORES = 8
TP = 4                                # TP group size
DP = D_INNER // TP                    # 512 channels per core
NDT = DP // 128                       # 4 d-tiles per core
BF16 = ml_dtypes.bfloat16

_CACHE = {}


def _build_program():
    import concourse.bass as bass
    import concourse.tile as tile
    from concourse import bacc, mybir

    F32, BF = mybir.dt.float32, mybir.dt.bfloat16
    ALU = mybir.AluOpType
    ACT = mybir.ActivationFunctionType

    nc = bacc.Bacc("TRN2", target_bir_lowering=False, debug=False,
                   num_devices=N_CORES)

    # ---- per-core external tensors ----
    xT = nc.dram_tensor("xT", [D_MODEL, L], BF, kind="ExternalInput")
    winT = nc.dram_tensor("winT", [D_MODEL, 2 * DP], BF, kind="ExternalInput")
    negrs = nc.dram_tensor("negrs", [1, 2 * DP], BF, kind="ExternalInput")
    c0rs = nc.dram_tensor("c0rs", [1, 2 * DP], BF, kind="ExternalInput")
    convw = nc.dram_tensor("convw", [DP, D_CONV], F32, kind="ExternalInput")
    convb = nc.dram_tensor("convb", [DP, 1], F32, kind="ExternalInput")
    xpwT = nc.dram_tensor("xpwT", [DP, DT_RANK + 2 * D_STATE], BF, kind="ExternalInput")
    dtwT = nc.dram_tensor("dtwT", [DT_RANK, DP], BF, kind="ExternalInput")
    dtb = nc.dram_tensor("dtb", [DP, 1], F32, kind="ExternalInput")
    Aneg = nc.dram_tensor("Aneg", [DP, D_STATE], F32, kind="ExternalInput")
    dskd = nc.dram_tensor("dskd", [DP, 128], BF, kind="ExternalInput")
    ident_in = nc.dram_tensor("ident", [128, 128], BF, kind="ExternalInput")
    owT = nc.dram_tensor("owT", [DP, D_MODEL], BF, kind="ExternalInput")
    out = nc.dram_tensor("out", [D_MODEL, L], F32, kind="ExternalOutput")

    NK = D_MODEL // 128               # 8 k-chunks
    NM = (2 * DP) // 128              # 8 m-chunks of in_proj output
    NTC = L // 512                    # 4 t-chunks of 512

    with tile.TileContext(nc) as tc:
        with tc.tile_pool(name="persist", bufs=1) as pp, \
             tc.tile_pool(name="dram", bufs=1, space="DRAM") as dram:

            # persistent SBUF tiles
            uT = [pp.tile([128, L + 4], BF, tag=f"uT{i}", name=f"uT{i}") for i in range(NDT)]
            zT = [pp.tile([128, L], BF, tag=f"zT{i}", name=f"zT{i}") for i in range(NDT)]
            ucT = [pp.tile([128, L], BF, tag=f"ucT{i}", name=f"ucT{i}") for i in range(NDT)]
            szT = [pp.tile([128, L], BF, tag=f"szT{i}", name=f"szT{i}") for i in range(NDT)]
            dtT = [pp.tile([128, L], BF, tag=f"dtT{i}", name=f"dtT{i}") for i in range(NDT)]
            dtuT = [pp.tile([128, L], BF, tag=f"dtuT{i}", name=f"dtuT{i}") for i in range(NDT)]
            ysg = [pp.tile([128, L], BF, tag=f"ysg{i}", name=f"ysg{i}") for i in range(NDT)]
            ident = pp.tile([128, 128], BF, tag="ident")
            nc.sync.dma_start(ident[:], ident_in.ap())
            dskd_sb, Aneg_sb, convw_sb, convb_sb, dtb_sb = [], [], [], [], []
            for i in range(NDT):
                rsl = slice(i * 128, (i + 1) * 128)
                t = pp.tile([128, 128], BF, name=f"dskd{i}")
                nc.sync.dma_start(t[:], dskd.ap()[rsl, :]); dskd_sb.append(t)
                t = pp.tile([128, D_STATE], F32, name=f"Aneg{i}")
                nc.sync.dma_start(t[:], Aneg.ap()[rsl, :]); Aneg_sb.append(t)
                t = pp.tile([128, D_CONV], F32, name=f"convw{i}")
                nc.sync.dma_start(t[:], convw.ap()[rsl, :]); convw_sb.append(t)
                t = pp.tile([128, 1], F32, name=f"convb{i}")
                nc.sync.dma_start(t[:], convb.ap()[rsl, :]); convb_sb.append(t)
                t = pp.tile([128, 1], F32, name=f"dtb{i}")
                nc.sync.dma_start(t[:], dtb.ap()[rsl, :]); dtb_sb.append(t)
            negrs_sb = pp.tile([1, 2 * DP], BF, tag="negrs")
            nc.sync.dma_start(negrs_sb[:], negrs.ap())
            c0rs_sb = pp.tile([1, 2 * DP], BF, tag="c0rs")
            nc.sync.dma_start(c0rs_sb[:], c0rs.ap())

            # DRAM scratch
            xdbl_part = dram.tile([DT_RANK + 2 * D_STATE, L], F32, tag="xdp")
            xdbl_red = dram.tile([DT_RANK + 2 * D_STATE, L], F32, tag="xdr")
            bc16 = dram.tile([2 * D_STATE, L], BF, tag="bc16")
            rstd_dram = dram.tile([1, L], F32, tag="rstdd")

            # zero the conv pad columns
            for i in range(NDT):
                nc.vector.memset(uT[i][:, 0:4], 0.0)

            # ---------- Phase A: load xT, stats, in_proj ----------
            with tc.tile_pool(name="xk", bufs=1) as xkp, \
                 tc.tile_pool(name="stat", bufs=1) as statp:
                xk = []
                for kc in range(NK):
                    t = xkp.tile([128, L], BF, tag=f"xk{kc}")
                    nc.sync.dma_start(t[:], xT.ap()[kc * 128:(kc + 1) * 128, :])
                    xk.append(t)

                ones = pp.tile([128, 1], BF, tag="ones")
                nc.vector.memset(ones[:], 1.0)

                # stats: S1 = colsum(x), S2 = colsum(x^2) via PE ones-reduce
                with tc.tile_pool(name="stps", bufs=1, space="PSUM") as stps, \
                     tc.tile_pool(name="sq", bufs=2) as sqp:
                    S1 = stps.tile([1, L], F32, tag="S1")
                    S2 = stps.tile([1, L], F32, tag="S2")
                    for kc in range(NK):
                        x2 = sqp.tile([128, L], BF, tag="x2")
                        nc.scalar.activation(x2[:], xk[kc][:], ACT.Square)
                        for t4 in range(NTC):
                            sl = slice(t4 * 512, (t4 + 1) * 512)
                            nc.tensor.matmul(S1[:, sl], ones[:], xk[kc][:, sl],
                                             start=(kc == 0), stop=(kc == NK - 1))
                            nc.tensor.matmul(S2[:, sl], ones[:], x2[:, sl],
                                             start=(kc == 0), stop=(kc == NK - 1))
                    # -> SBUF
                    s1f = statp.tile([1, L], F32, tag="s1f")
                    s2f = statp.tile([1, L], F32, tag="st", bufs=3)
                    nc.scalar.activation(s1f[:], S1[:], ACT.Copy)
                    nc.scalar.activation(s2f[:], S2[:], ACT.Copy)

                # rstd = exp(-0.5*log(var+eps)); sd = exp(+0.5*log(var+eps))
                mu2 = statp.tile([1, L], F32, tag="st", bufs=3)
                nc.scalar.activation(mu2[:], s1f[:], ACT.Square, scale=1.0 / D_MODEL)
                var = statp.tile([1, L], F32, tag="st", bufs=3)
                nc.vector.scalar_tensor_tensor(var[:], s2f[:], 1.0 / D_MODEL, mu2[:],
                                               ALU.mult, ALU.subtract)
                epsb = statp.tile([1, 1], F32, tag="epsb")
                nc.vector.memset(epsb[:], EPS)
                lv = statp.tile([1, L], F32, tag="st", bufs=3)
                nc.scalar.activation(lv[:], var[:], ACT.Ln, bias=epsb[:])
                rstd_row = statp.tile([1, L], F32, tag="st", bufs=3)
                nc.scalar.activation(rstd_row[:], lv[:], ACT.Exp, scale=-0.5)
                sd16 = pp.tile([1, L], BF, tag="sd16")
                nc.scalar.activation(sd16[:], lv[:], ACT.Exp, scale=0.5)
                s1_16 = pp.tile([1, L], BF, tag="s1_16")
                nc.vector.tensor_copy(s1_16[:], s1f[:])
                # broadcast rstd to all partitions via DRAM bounce
                nc.sync.dma_start(rstd_dram[:], rstd_row[:])
                rstd_bc = pp.tile([128, L], F32, tag="rstbc")
                nc.sync.dma_start(rstd_bc[:], rstd_dram[0, :].partition_broadcast(128))

                # in_proj
                with tc.tile_pool(name="wln", bufs=4) as wp, \
                     tc.tile_pool(name="xzps", bufs=2, space="PSUM") as xzps:
                    for mc in range(NM):
                        ps = xzps.tile([128, L], F32, tag="xz")
                        for kc in range(NK):
                            w = wp.tile([128, 128], BF, tag="w")
                            nc.sync.dma_start(
                                w[:], winT.ap()[kc * 128:(kc + 1) * 128,
                                                mc * 128:(mc + 1) * 128])
                            for t4 in range(NTC):
                                sl = slice(t4 * 512, (t4 + 1) * 512)
                                nc.tensor.matmul(ps[:, sl], w[:], xk[kc][:, sl],
                                                 start=(kc == 0), stop=False)
                        for t4 in range(NTC):
                            sl = slice(t4 * 512, (t4 + 1) * 512)
                            nc.tensor.matmul(ps[:, sl],
                                             negrs_sb[:, mc * 128:(mc + 1) * 128],
                                             s1_16[:, sl], start=False, stop=False)
                            nc.tensor.matmul(ps[:, sl],
                                             c0rs_sb[:, mc * 128:(mc + 1) * 128],
                                             sd16[:, sl], start=False, stop=True)
                        if mc < NDT:
                            nc.vector.tensor_tensor(uT[mc][:, 4:4 + L], ps[:],
                                                    rstd_bc[:], ALU.mult)
                        else:
                            nc.vector.tensor_tensor(zT[mc - NDT][:], ps[:],
                                                    rstd_bc[:], ALU.mult)

            # ---------- Phase B ----------
            # conv: 4 shifted taps scaled on ACT, summed on PE, SiLU evac
            with tc.tile_pool(name="cvps", bufs=2, space="PSUM") as cvps, \
                 tc.tile_pool(name="taps", bufs=2) as tapp:
                for i in range(NDT):
                    psc = cvps.tile([128, L], F32, tag="cv")
                    cks = []
                    for k in range(D_CONV):
                        ck = tapp.tile([128, L], BF, tag=f"ck{k}")
                        nc.scalar.activation(ck[:], uT[i][:, 1 + k:1 + k + L],
                                             ACT.Copy,
                                             scale=convw_sb[i][:, k:k + 1])
                        cks.append(ck)
                    for t4 in range(NTC):
                        sl = slice(t4 * 512, (t4 + 1) * 512)
                        for k in range(D_CONV):
                            nc.tensor.matmul(psc[:, sl], ident[:], cks[k][:, sl],
                                             start=(k == 0), stop=(k == D_CONV - 1))
                    nc.scalar.activation(ucT[i][:], psc[:], ACT.Silu,
                                         bias=convb_sb[i][:])
                    # silu(z) too
                    nc.scalar.activation(szT[i][:], zT[i][:], ACT.Silu)
            # x_proj partial + AllReduce
            NXP = DT_RANK + 2 * D_STATE
            with tc.tile_pool(name="xpps", bufs=1, space="PSUM") as xpps, \
                 tc.tile_pool(name="xpw", bufs=1) as xpwp:
                psx = xpps.tile([NXP, L], F32, tag="xp")
                xpw_sb = []
                for i in range(NDT):
                    t = xpwp.tile([128, NXP], BF, name=f"xpw{i}")
                    nc.sync.dma_start(t[:], xpwT.ap()[i * 128:(i + 1) * 128, :])
                    xpw_sb.append(t)
                for i in range(NDT):
                    for t4 in range(NTC):
                        sl = slice(t4 * 512, (t4 + 1) * 512)
                        nc.tensor.matmul(psx[:, sl], xpw_sb[i][:],
                                         ucT[i][:, sl],
                                         start=(i == 0), stop=(i == NDT - 1))
                xdbl_sb = xpwp.tile([NXP, L], F32, tag="xdbl")
                nc.scalar.activation(xdbl_sb[:], psx[:], ACT.Copy)
                nc.sync.dma_start(xdbl_part[:], xdbl_sb[:])

            nc.gpsimd.collective_compute(
                "AllReduce", ALU.add,
                replica_groups=[[0, 1, 2, 3], [4, 5, 6, 7]],
                ins=[xdbl_part[:].opt()],
                outs=[xdbl_red[:].opt()],
            )

            # dtr (bf16) and B/C rows (bf16, bounced to DRAM for broadcasts)
            with tc.tile_pool(name="dtr", bufs=1) as dtrp:
                dtr_f = dtrp.tile([DT_RANK, L], F32, tag="dtrf")
                nc.sync.dma_start(dtr_f[:], xdbl_red[0:DT_RANK, :])
                dtr16 = dtrp.tile([DT_RANK, L], BF, tag="dtr16")
                nc.vector.tensor_copy(dtr16[:], dtr_f[:])
                bc_f = dtrp.tile([2 * D_STATE, L], F32, tag="bcf")
                nc.sync.dma_start(bc_f[:], xdbl_red[DT_RANK:NXP, :])
                bc_sb = dtrp.tile([2 * D_STATE, L], BF, tag="bcs")
                nc.vector.tensor_copy(bc_sb[:], bc_f[:])
                nc.sync.dma_start(bc16[:], bc_sb[:])

                # dt_proj + softplus + dtu
                with tc.tile_pool(name="dtps", bufs=2, space="PSUM") as dtps, \
                     tc.tile_pool(name="dtw", bufs=1) as dtwp:
                    dtw_sb = dtwp.tile([DT_RANK, DP], BF, tag="dtw")
                    nc.sync.dma_start(dtw_sb[:], dtwT.ap())
                    etile = dtwp.tile([128, L], F32, tag="et")
                    for i in range(NDT):
                        psd = dtps.tile([128, L], F32, tag="dt")
                        for t4 in range(NTC):
                            sl = slice(t4 * 512, (t4 + 1) * 512)
                            nc.tensor.matmul(psd[:, sl],
                                             dtw_sb[:, i * 128:(i + 1) * 128],
                                             dtr16[:, sl], start=True, stop=True)
                        dsl = slice(i * 128, (i + 1) * 128)
                        nc.scalar.activation(etile[:], psd[:], ACT.Exp,
                                             bias=dtb_sb[i][:])
                        nc.scalar.activation(dtT[i][:], etile[:], ACT.Ln, bias=1.0)
                        nc.vector.tensor_tensor(dtuT[i][:], dtT[i][:],
                                                ucT[i][:], ALU.mult)

            # ---------- Phase C: selective scan ----------
            with tc.tile_pool(name="ysps", bufs=1, space="PSUM") as ysps, \
                 tc.tile_pool(name="scw", bufs=2) as scw, \
                 tc.tile_pool(name="bcw", bufs=2) as bcw:
                for pair in range(NDT // 2):
                    ys = []
                    for j in range(2):
                        i = pair * 2 + j
                        ps = ysps.tile([128, L], F32, tag=f"ys{j}")
                        for t4 in range(NTC):
                            sl = slice(t4 * 512, (t4 + 1) * 512)
                            nc.tensor.matmul(ps[:, sl], dskd_sb[i][:],
                                             ucT[i][:, sl], start=True, stop=False)
                        ys.append(ps)
                    for n in range(D_STATE):
                        bbc = bcw.tile([128, L], BF, tag="bbc")
                        nc.sync.dma_start(bbc[:], bc16[n, :].partition_broadcast(128))
                        cbc = bcw.tile([128, L], BF, tag="cbc")
                        nc.sync.dma_start(
                            cbc[:], bc16[D_STATE + n, :].partition_broadcast(128))
                        for j in range(2):
                            i = pair * 2 + j
                            dsl = slice(i * 128, (i + 1) * 128)
                            dA = scw.tile([128, L], F32, tag="dA")
                            nc.scalar.activation(dA[:], dtT[i][:], ACT.Exp,
                                                 scale=Aneg_sb[i][:, n:n + 1])
                            dBu = scw.tile([128, L], BF, tag="dBu", bufs=3)
                            nc.vector.tensor_tensor(dBu[:], dtuT[i][:], bbc[:],
                                                    ALU.mult)
                            h = scw.tile([128, L], BF, tag="h", bufs=3)
                            nc.vector.tensor_tensor_scan(h[:], dA[:], dBu[:], 0.0,
                                                         ALU.mult, ALU.add)
                            hC = scw.tile([128, L], BF, tag="hC", bufs=3)
                            nc.vector.tensor_tensor(hC[:], h[:], cbc[:], ALU.mult)
                            last = (n == D_STATE - 1)
                            for t4 in range(NTC):
                                sl = slice(t4 * 512, (t4 + 1) * 512)
                                nc.tensor.matmul(ys[j][:, sl], ident[:],
                                                 hC[:, sl], start=False, stop=last)
                    for j in range(2):
                        i = pair * 2 + j
                        nc.vector.tensor_tensor(ysg[i][:], ys[j][:], szT[i][:],
                                                ALU.mult)

            # ---------- Phase D: out_proj ----------
            with tc.tile_pool(name="ops", bufs=4, space="PSUM") as ops, \
                 tc.tile_pool(name="ow", bufs=1) as owp, \
                 tc.tile_pool(name="oev", bufs=4) as oevp:
                ow_sb = []
                for i in range(NDT):
                    t = owp.tile([128, D_MODEL], BF, name=f"ow{i}")
                    nc.sync.dma_start(t[:], owT.ap()[i * 128:(i + 1) * 128, :])
                    ow_sb.append(t)
                for mc in range(D_MODEL // 128):
                    for t4 in range(NTC):
                        sl = slice(t4 * 512, (t4 + 1) * 512)
                        po = ops.tile([128, 512], F32, tag="po")
                        for i in range(NDT):
                            nc.tensor.matmul(po[:],
                                             ow_sb[i][:, mc * 128:(mc + 1) * 128],
                                             ysg[i][:, sl],
                                             start=(i == 0), stop=(i == NDT - 1))
                        oe = oevp.tile([128, 512], F32, tag="oe")
                        nc.scalar.activation(oe[:], po[:], ACT.Copy)
                        nc.sync.dma_start(
                            out.ap()[mc * 128:(mc + 1) * 128, sl], oe[:])

    nc.compile()
    return nc


def _prep_inputs(x, ln_w, ln_b, in_proj_w, conv_w, conv_b, x_proj_w,
                 dt_proj_w, dt_proj_b, A_log, Dskip, out_proj_w):
    """Host-side shard + transpose + dtype prep. Returns list of 8 in_maps."""
    f32 = np.float32
    x = np.asarray(x, f32)
    ln_w = np.asarray(ln_w, f32); ln_b = np.asarray(ln_b, f32)
    W = np.asarray(in_proj_w, f32)
    W_eff = W * ln_w[None, :]
    c0 = W @ ln_b                                  # [2*D_INNER]
    rs = W_eff.sum(axis=1)                         # [2*D_INNER]
    A = -np.exp(np.asarray(A_log, f32))            # [D_INNER, 16]
    conv_w = np.asarray(conv_w, f32).reshape(D_INNER, D_CONV)
    conv_b = np.asarray(conv_b, f32)
    xpw = np.asarray(x_proj_w, f32)                # [96, D_INNER]
    dtw = np.asarray(dt_proj_w, f32)               # [D_INNER, 64]
    dtb = np.asarray(dt_proj_b, f32)
    Dsk = np.asarray(Dskip, f32)
    Ow = np.asarray(out_proj_w, f32)               # [D_MODEL, D_INNER]
    ident = np.eye(128, dtype=BF16)

    in_maps = []
    for c in range(N_CORES):
        b, g = divmod(c, TP)
        dsl = slice(g * DP, (g + 1) * DP)
        u_rows = slice(g * DP, (g + 1) * DP)
        z_rows = slice(D_INNER + g * DP, D_INNER + (g + 1) * DP)
        winT = np.concatenate([W_eff[u_rows].T, W_eff[z_rows].T], axis=1)  # [1024, 1024]
        negrs_c = -np.concatenate([rs[u_rows], rs[z_rows]]) / D_MODEL
        c0_c = np.concatenate([c0[u_rows], c0[z_rows]])
        dskd = np.zeros((DP, 128), BF16)
        for i in range(NDT):
            blk = np.diag(Dsk[g * DP + i * 128: g * DP + (i + 1) * 128])
            dskd[i * 128:(i + 1) * 128, :] = blk.astype(BF16)
        in_maps.append({
            "xT": np.ascontiguousarray(x[b].T).astype(BF16),
            "winT": winT.astype(BF16),
            "negrs": negrs_c[None, :].astype(BF16),
            "c0rs": c0_c[None, :].astype(BF16),
            "convw": np.ascontiguousarray(conv_w[dsl]),
            "convb": conv_b[dsl][:, None].copy(),
            "xpwT": np.ascontiguousarray(xpw[:, dsl].T).astype(BF16),
            "dtwT": np.ascontiguousarray(dtw[dsl].T).astype(BF16),
            "dtb": dtb[dsl][:, None].copy(),
            "Aneg": np.ascontiguousarray(A[dsl]),
            "dskd": dskd,
            "ident": ident,
            "owT": np.ascontiguousarray(Ow[:, dsl].T).astype(BF16),
        })
    return in_maps


def kernel(**inputs):
    from concourse.bass_utils import run_bass_kernel_spmd

    if "nc" not in _CACHE:
        _CACHE["nc"] = _build_program()
    nc = _CACHE["nc"]

    in_maps = _prep_inputs(**inputs)
    res = run_bass_kernel_spmd(nc, in_maps, core_ids=list(range(N_CORES)))

    x = np.asarray(inputs["x"], np.float32)
    out = np.empty((B, L, D_MODEL), np.float32)
    for b in range(B):
        acc = res.results[4 * b]["out"].copy()
        for g in range(1, TP):
            acc += res.results[4 * b + g]["out"]
        out[b] = acc.T + x[b]
    return out
